# revision 17
# baseline (speedup 1.0000x reference)
"""MoE (base FFN + top-2-of-8 expert FFNs) on 8 TRN2 NeuronCores.

Strategy (expert-parallel):
  - Routing (softmax over 8 experts, top-2, renormalize) is computed on
    host with jax-CPU, mirroring the reference computation exactly.
  - Core e owns expert e: host gathers the tokens routed to expert e
    (padded to capacity C = roundup(max_e n_e, 64)), core e runs the
    expert FFN on them and scales by the renormalized routing weight.
  - Core e also runs the base FFN for tokens [512e, 512(e+1)).
  - Host scatters expert outputs back (scatter-add) on top of base.

Device compute in bf16 with fp32 PSUM accumulation; activations stay in
[feature, token] layout so both FFN matmuls chain without transposes.
Token/scale loads are issued from the sync engine, weight-tile loads
from the scalar engine, and output stores from gpsimd, so no DMA stream
head-of-line-blocks another.
"""

import numpy as np
import ml_dtypes

import concourse.bass as bass
import concourse.mybir as mybir
import concourse.tile as tile
from concourse import bacc
from concourse.bass_utils import run_bass_kernel_spmd
from concourse.tile_rust import add_dep_helper

P = 128
B, S, H, F, E = 2, 2048, 1024, 4096, 8
T = B * S
NB = T // 8  # base-FFN tokens per core
TOP_K = 2
BETA = 1.0

F32 = mybir.dt.float32
CDT = mybir.dt.bfloat16  # compute dtype on the tensor engine
NP_CDT = ml_dtypes.bfloat16

KA = H // P   # 8  k-subtiles contracting H
FB = F // P   # 32 output blocks of F
KB = F // P   # 32 k-subtiles contracting F
HB = H // P   # 8  output blocks of H
CHUNK = 512   # matmul moving free dim / PSUM bank width


def _chunks(n):
    out = []
    c0 = 0
    while c0 < n:
        out.append((c0, min(CHUNK, n - c0)))
        c0 += CHUNK
    return out


def _stage(nc, wpool, pspool, wtag, w_d, x_s, cts, evict, wt0=None, cm_last=False):
    """One matmul stage: out[ob] = evict(sum_k w[ob,k].T @ x[k]) per chunk.

    w_d: DRAM [P, OB, K, 128]; x_s: SBUF [P, K, n_cols].
    wt0: optional pre-loaded weight tile for ob==0.
    cm_last: run the final output block chunk-major (each chunk finishes its
    full contraction and evicts before the next starts), so the kernel's very
    last eviction is the small tail chunk instead of three back-to-back
    512-wide ones. Costs re-loading that block's weights once per chunk.

    Weight tiles all ride the scalar queue (hardware DGE). The gpsimd queue
    is software DGE and too slow for the bulk weight stream — splitting
    weights onto it starves the PE and drops the HAM clock gate.
    """
    OB, K = w_d.shape[1], w_d.shape[2]
    for ob in range(OB):
        if cm_last and ob == OB - 1 and len(cts) > 1:
            for c0, cn in cts:
                ps = pspool.tile([P, CHUNK], F32, name="ps")
                wts = wpool.tile([P, K, P], CDT, name=wtag)
                for k0 in range(0, K, 8):
                    nc.scalar.dma_start(
                        out=wts[:, k0 : k0 + 8], in_=w_d[:, ob, k0 : k0 + 8]
                    )
                for k in range(K):
                    nc.tensor.matmul(
                        ps[:, :cn],
                        wts[:, k],
                        x_s[:, k, c0 : c0 + cn],
                        start=(k == 0),
                        stop=(k == K - 1),
                    )
                evict(ob, ps, c0, cn)
            continue
        if ob == 0 and wt0 is not None:
            wt = wt0
        else:
            wt = wpool.tile([P, K, P], CDT, name=wtag)
            nc.scalar.dma_start(out=wt[:], in_=w_d[:, ob])
        pss = [
            (pspool.tile([P, CHUNK], F32, name="ps"), c0, cn) for c0, cn in cts
        ]
        for k in range(K):
            for ps, c0, cn in pss:
                nc.tensor.matmul(
                    ps[:, :cn],
                    wt[:, k],
                    x_s[:, k, c0 : c0 + cn],
                    start=(k == 0),
                    stop=(k == K - 1),
                )
        for ps, c0, cn in pss:
            evict(ob, ps, c0, cn)


def _build(C):
    """Build the per-core SPMD program for moe capacity C (multiple of 64)."""
    nc = bacc.Bacc(None, target_bir_lowering=False, debug=False)
    act_silu = mybir.ActivationFunctionType.Silu
    with tile.TileContext(nc) as tc:
        with tc.tile_pool(name="dram", bufs=1, space="DRAM") as dram:
            kw = dict(kind="ExternalInput", uniquify=False)
            xg = dram.tile((P, KA, C), CDT, name="xg", **kw)
            wg = dram.tile((P, C), F32, name="wg", **kw)
            w1 = dram.tile((P, FB, KA, P), CDT, name="w1", **kw)
            w2 = dram.tile((P, HB, KB, P), CDT, name="w2", **kw)
            xb = dram.tile((P, KA, NB), CDT, name="xb", **kw)
            b1 = dram.tile((P, FB, KA, P), CDT, name="b1", **kw)
            b2 = dram.tile((P, HB, KB, P), CDT, name="b2", **kw)
            ymoe = dram.tile(
                (P, HB, C), CDT, name="ymoe", kind="ExternalOutput", uniquify=False
            )
            ybase = dram.tile(
                (P, HB, NB), CDT, name="ybase", kind="ExternalOutput", uniquify=False
            )
            with (
                tc.tile_pool(name="res", bufs=1) as res,
                tc.tile_pool(name="wa", bufs=8) as wa,
                tc.tile_pool(name="wb", bufs=4) as wb,
                tc.tile_pool(name="ps", bufs=8, space="PSUM") as ps,
                tc.tile_pool(name="yo", bufs=4) as yo,
            ):
                cts_m = _chunks(C)
                cts_b = _chunks(NB)

                # Startup-critical loads go first on their queues: the first
                # base1 weight tile (scalar q) and the base tokens (sync q).
                # Warm-up scratch memsets lead the gpsimd queue so the junk
                # matmuls can issue immediately after the ~7us framework
                # preamble.
                wlhs = res.tile([P, P], CDT, name="wlhs")
                nc.gpsimd.memset(wlhs[:], 0.0)
                wrhs = res.tile([P, CHUNK], CDT, name="wrhs")
                nc.gpsimd.memset(wrhs[:], 0.0)

                # Startup-critical loads, finely sliced and spread over the
                # DMA queues so the first real matmul's inputs (b1 k-slice 0
                # + xb k0:2) land as early as possible after the preamble.
                wt_b1 = wa.tile([P, KA, P], CDT, name="wa")
                for k0 in range(0, KA, 2):
                    nc.scalar.dma_start(
                        out=wt_b1[:, k0 : k0 + 2], in_=b1[:, 0, k0 : k0 + 2]
                    )
                xb_s = res.tile([P, KA, NB], CDT, name="xb_s")
                xb_engines = [nc.sync, nc.gpsimd, nc.sync, nc.gpsimd]
                for i, k0 in enumerate(range(0, KA, 2)):
                    xb_engines[i].dma_start(
                        out=xb_s[:, k0 : k0 + 2], in_=xb[:, k0 : k0 + 2]
                    )

                # PE warm-up: junk matmuls bridge from the end of the
                # framework preamble (~7.5us) to when the first real inputs
                # land (~11-12us), keeping the PE busy the whole time so the
                # HAM clock gate opens (~3.4us of sustained busy) and stays
                # open into the real stream.
                for _ in range(10):
                    wps = ps.tile([P, CHUNK], F32, name="ps")
                    nc.tensor.matmul(wps[:], wlhs[:], wrhs[:], start=True, stop=True)

                h2 = res.tile([P, KB, NB], CDT, name="h2")

                base1_marker = []

                def ev_base1(ob, psum, c0, cn):
                    act = nc.scalar.activation(
                        h2[:, ob, c0 : c0 + cn], psum[:, :cn], act_silu
                    )
                    if ob == 1:
                        base1_marker.append(act)

                _stage(nc, wa, ps, "wa", b1, xb_s, cts_b, ev_base1, wt0=wt_b1)

                # expert tokens: loaded during base compute; explicitly
                # gated on early base1 progress so this 2.2MB transfer never
                # competes with the startup-critical xb/b1 loads
                xg_s = res.tile([P, KA, C], CDT, name="xg_s")
                for k in range(0, KA, 2):
                    dma = nc.sync.dma_start(out=xg_s[:, k : k + 2], in_=xg[:, k : k + 2])
                    add_dep_helper(
                        dma.ins,
                        base1_marker[0].ins,
                        reason="defer xg load past startup window",
                    )

                # Output DMAs rotate over gpsimd/sync; scalar stays
                # dedicated to the weight-tile stream so a queued output
                # never delays the next weight load.
                out_engines = [nc.gpsimd, nc.sync]
                ev_n = [0]

                def _out_dma(dst, o, cn):
                    eng = out_engines[ev_n[0] % 2]
                    ev_n[0] += 1
                    eng.dma_start(out=dst, in_=o[:, :cn])

                def ev_base2(ob, psum, c0, cn):
                    o = yo.tile([P, CHUNK], CDT, name="yo")
                    nc.vector.tensor_copy(out=o[:, :cn], in_=psum[:, :cn])
                    _out_dma(ybase[:, ob, c0 : c0 + cn], o, cn)

                _stage(nc, wb, ps, "wb", b2, h2, cts_b, ev_base2)

                h1 = res.tile([P, KB, C], CDT, name="h1")

                def ev_moe1(ob, psum, c0, cn):
                    nc.scalar.activation(
                        h1[:, ob, c0 : c0 + cn], psum[:, :cn], act_silu
                    )

                _stage(nc, wa, ps, "wa", w1, xg_s, cts_m, ev_moe1)

                wg_s = res.tile([P, C], F32, name="wg_s")
                nc.sync.dma_start(out=wg_s[:], in_=wg[:])

                def ev_moe2(ob, psum, c0, cn):
                    o = yo.tile([P, CHUNK], CDT, name="yo")
                    nc.vector.tensor_tensor(
                        out=o[:, :cn],
                        in0=psum[:, :cn],
                        in1=wg_s[:, c0 : c0 + cn],
                        op=mybir.AluOpType.mult,
                    )
                    _out_dma(ymoe[:, ob, c0 : c0 + cn], o, cn)

                _stage(nc, wb, ps, "wb", w2, h1, cts_m, ev_moe2, cm_last=True)
    nc.compile()
    return nc


_BUILD_CACHE = {}


def _get_program(C):
    if C not in _BUILD_CACHE:
        _BUILD_CACHE[C] = _build(C)
    return _BUILD_CACHE[C]


def _routing(x, router_w):
    """Replicate the reference router bit-for-bit on jax CPU."""
    import jax
    import jax.numpy as jnp

    cpu = jax.devices("cpu")[0]

    def _route(xj, rj):
        logits = xj @ rj
        probs = jax.nn.softmax(logits, axis=-1)
        top_w, top_i = jax.lax.top_k(probs, TOP_K)
        top_w = top_w / jnp.sum(top_w, axis=-1, keepdims=True)
        return top_w, top_i

    with jax.default_device(cpu):
        top_w, top_i = jax.jit(_route)(jnp.asarray(x), jnp.asarray(router_w))
        top_w = np.asarray(top_w)
        top_i = np.asarray(top_i)
    return top_w, top_i


def _as_pkc(w, kb, nb):
    # [K, N] -> [P, nblocks, kblocks, 128]: w[k*128+p, n*128+c] -> [p, n, k, c]
    return np.ascontiguousarray(w.reshape(kb, P, nb, P).transpose(1, 2, 0, 3))


def _as_pit(xt):
    # [R, N] -> [P, R//128, N]: xt[i*128+p, t] -> [p, i, t]
    r, n = xt.shape
    return np.ascontiguousarray(xt.reshape(r // P, P, n).transpose(1, 0, 2))


def _from_pit(y):
    # [P, R//128, N] -> [N, R]
    p, i, n = y.shape
    return y.transpose(2, 1, 0).reshape(n, i * p)


def kernel(hidden_states, router_w, base_w1, base_w2, exp_w1, exp_w2):
    x = np.ascontiguousarray(hidden_states.reshape(T, H), dtype=np.float32)
    top_w, top_i = _routing(x, np.asarray(router_w, dtype=np.float32))

    # per-expert token lists
    idx = []
    wts = []
    for e in range(E):
        rows, slots = np.nonzero(top_i == e)
        idx.append(rows)
        wts.append(top_w[rows, slots].astype(np.float32))
    nmax = max(len(r) for r in idx)
    C = max(((nmax + 63) // 64) * 64, 64)

    nc = _get_program(C)

    xT_c = np.ascontiguousarray(x.T).astype(NP_CDT)  # [H, T]

    b1_dev = _as_pkc(np.asarray(base_w1, np.float32).astype(NP_CDT), KA, FB)
    b2_dev = _as_pkc(np.asarray(base_w2, np.float32).astype(NP_CDT), KB, HB)

    in_maps = []
    for e in range(E):
        n_e = len(idx[e])
        xg_full = np.zeros((H, C), dtype=NP_CDT)
        xg_full[:, :n_e] = xT_c[:, idx[e]]
        wg_full = np.zeros((C,), dtype=np.float32)
        wg_full[:n_e] = wts[e]
        in_maps.append(
            {
                "xg": _as_pit(xg_full),
                "wg": np.ascontiguousarray(np.broadcast_to(wg_full, (P, C))),
                "w1": _as_pkc(
                    np.asarray(exp_w1[e], np.float32).astype(NP_CDT), KA, FB
                ),
                "w2": _as_pkc(
                    np.asarray(exp_w2[e], np.float32).astype(NP_CDT), KB, HB
                ),
                "xb": _as_pit(xT_c[:, e * NB : (e + 1) * NB]),
                "b1": b1_dev,
                "b2": b2_dev,
            }
        )

    res = run_bass_kernel_spmd(nc, in_maps, core_ids=list(range(8)))

    out = np.empty((T, H), dtype=np.float32)
    for e in range(E):
        out[e * NB : (e + 1) * NB] = _from_pit(
            res.results[e]["ybase"].astype(np.float32)
        )
    for e in range(E):
        n_e = len(idx[e])
        ym = _from_pit(res.results[e]["ymoe"].astype(np.float32))[:n_e]
        out[idx[e]] += BETA * ym
    return out.reshape(B, S, H)



# revision 19
# speedup vs baseline: 1.1860x; 1.1860x over previous
"""MoE (base FFN + top-2-of-8 expert FFNs) on 8 TRN2 NeuronCores.

Strategy (expert-parallel):
  - Routing (softmax over 8 experts, top-2, renormalize) is computed on
    host with jax-CPU, mirroring the reference computation exactly.
  - Core e owns expert e: host gathers the tokens routed to expert e
    (padded to capacity C = roundup(max_e n_e, 64)), core e runs the
    expert FFN on them and scales by the renormalized routing weight.
  - Core e also runs the base FFN for tokens [512e, 512(e+1)).
  - Host scatters expert outputs back (scatter-add) on top of base.

Device compute in bf16 with fp32 PSUM accumulation (bf16 outputs);
activations stay in [feature, token] layout so both FFN matmuls chain
without transposes. The weight-tile stream owns the scalar queue
(hardware DGE), token loads ride sync (+gpsimd for the startup xb
slices), and output stores rotate gpsimd/sync, so no DMA stream
head-of-line-blocks another. A junk-matmul bridge keeps the PE busy
from the end of the ~7us framework preamble until the first real
inputs land, so the HAM clock gate opens once and stays open.
"""

import numpy as np
import ml_dtypes

import concourse.bass as bass
import concourse.mybir as mybir
import concourse.tile as tile
from concourse import bacc
from concourse.bass_utils import run_bass_kernel_spmd
from concourse.tile_rust import add_dep_helper

P = 128
B, S, H, F, E = 2, 2048, 1024, 4096, 8
T = B * S
NB = T // 8  # base-FFN tokens per core
TOP_K = 2
BETA = 1.0

F32 = mybir.dt.float32
CDT = mybir.dt.bfloat16  # compute dtype on the tensor engine
NP_CDT = ml_dtypes.bfloat16

KA = H // P   # 8  k-subtiles contracting H
FB = F // P   # 32 output blocks of F
KB = F // P   # 32 k-subtiles contracting F
HB = H // P   # 8  output blocks of H
CHUNK = 512   # matmul moving free dim / PSUM bank width


def _chunks(n):
    out = []
    c0 = 0
    while c0 < n:
        out.append((c0, min(CHUNK, n - c0)))
        c0 += CHUNK
    return out


def _stage(nc, wpool, pspool, wtag, w_d, x_s, cts, evict, wt0=None, cm_last=False):
    """One matmul stage: out[ob] = evict(sum_k w[ob,k].T @ x[k]) per chunk.

    w_d: DRAM [P, OB, K, 128]; x_s: SBUF [P, K, n_cols].
    wt0: optional pre-loaded weight tile for ob==0.
    cm_last: run the final output block chunk-major (each chunk finishes its
    full contraction and evicts before the next starts), so the kernel's very
    last eviction is the small tail chunk instead of three back-to-back
    512-wide ones. Costs re-loading that block's weights once per chunk.

    Weight tiles all ride the scalar queue (hardware DGE). The gpsimd queue
    is software DGE and too slow for the bulk weight stream — splitting
    weights onto it starves the PE and drops the HAM clock gate.
    """
    OB, K = w_d.shape[1], w_d.shape[2]
    for ob in range(OB):
        if cm_last and ob == OB - 1 and len(cts) > 1:
            for c0, cn in cts:
                ps = pspool.tile([P, CHUNK], F32, name="ps")
                wts = wpool.tile([P, K, P], CDT, name=wtag)
                for k0 in range(0, K, 8):
                    nc.scalar.dma_start(
                        out=wts[:, k0 : k0 + 8], in_=w_d[:, ob, k0 : k0 + 8]
                    )
                for k in range(K):
                    nc.tensor.matmul(
                        ps[:, :cn],
                        wts[:, k],
                        x_s[:, k, c0 : c0 + cn],
                        start=(k == 0),
                        stop=(k == K - 1),
                    )
                evict(ob, ps, c0, cn)
            continue
        if ob == 0 and wt0 is not None:
            wt = wt0
        else:
            wt = wpool.tile([P, K, P], CDT, name=wtag)
            nc.scalar.dma_start(out=wt[:], in_=w_d[:, ob])
        pss = [
            (pspool.tile([P, CHUNK], F32, name="ps"), c0, cn) for c0, cn in cts
        ]
        for k in range(K):
            for ps, c0, cn in pss:
                nc.tensor.matmul(
                    ps[:, :cn],
                    wt[:, k],
                    x_s[:, k, c0 : c0 + cn],
                    start=(k == 0),
                    stop=(k == K - 1),
                )
        for ps, c0, cn in pss:
            evict(ob, ps, c0, cn)


def _build(C):
    """Build the per-core SPMD program for moe capacity C (multiple of 64)."""
    nc = bacc.Bacc(None, target_bir_lowering=False, debug=False)
    act_silu = mybir.ActivationFunctionType.Silu
    with tile.TileContext(nc) as tc:
        with tc.tile_pool(name="dram", bufs=1, space="DRAM") as dram:
            kw = dict(kind="ExternalInput", uniquify=False)
            xg = dram.tile((P, KA, C), CDT, name="xg", **kw)
            wg = dram.tile((P, C), F32, name="wg", **kw)
            w1 = dram.tile((P, FB, KA, P), CDT, name="w1", **kw)
            w2 = dram.tile((P, HB, KB, P), CDT, name="w2", **kw)
            xb = dram.tile((P, KA, NB), CDT, name="xb", **kw)
            b1 = dram.tile((P, FB, KA, P), CDT, name="b1", **kw)
            b2 = dram.tile((P, HB, KB, P), CDT, name="b2", **kw)
            ymoe = dram.tile(
                (P, HB, C), CDT, name="ymoe", kind="ExternalOutput", uniquify=False
            )
            ybase = dram.tile(
                (P, HB, NB), CDT, name="ybase", kind="ExternalOutput", uniquify=False
            )
            with (
                tc.tile_pool(name="res", bufs=1) as res,
                tc.tile_pool(name="wa", bufs=8) as wa,
                tc.tile_pool(name="wb", bufs=4) as wb,
                tc.tile_pool(name="ps", bufs=8, space="PSUM") as ps,
                tc.tile_pool(name="yo", bufs=4) as yo,
            ):
                cts_m = _chunks(C)
                cts_b = _chunks(NB)

                # Warm-up scratch memsets lead the gpsimd queue so the junk
                # matmuls can issue immediately after the ~7us framework
                # preamble.
                wlhs = res.tile([P, P], CDT, name="wlhs")
                nc.gpsimd.memset(wlhs[:], 0.0)
                wrhs = res.tile([P, CHUNK], CDT, name="wrhs")
                nc.gpsimd.memset(wrhs[:], 0.0)

                # Startup-critical loads, finely sliced and spread over the
                # DMA queues so the first real matmul's inputs (b1 k-slice 0
                # + xb k0:2) land as early as possible after the preamble.
                wt_b1 = wa.tile([P, KA, P], CDT, name="wa")
                for k0 in range(0, KA, 2):
                    nc.scalar.dma_start(
                        out=wt_b1[:, k0 : k0 + 2], in_=b1[:, 0, k0 : k0 + 2]
                    )
                xb_s = res.tile([P, KA, NB], CDT, name="xb_s")
                xb_engines = [nc.sync, nc.gpsimd, nc.sync, nc.gpsimd]
                for i, k0 in enumerate(range(0, KA, 2)):
                    xb_engines[i].dma_start(
                        out=xb_s[:, k0 : k0 + 2], in_=xb[:, k0 : k0 + 2]
                    )

                # PE warm-up: junk matmuls bridge from the end of the
                # framework preamble (~7.5us) to when the first real inputs
                # land (~11-12us), keeping the PE busy the whole time so the
                # HAM clock gate opens (~3.4us of sustained busy) and stays
                # open into the real stream.
                for _ in range(10):
                    wps = ps.tile([P, CHUNK], F32, name="ps")
                    nc.tensor.matmul(wps[:], wlhs[:], wrhs[:], start=True, stop=True)

                h2 = res.tile([P, KB, NB], CDT, name="h2")

                base1_marker = []

                def ev_base1(ob, psum, c0, cn):
                    act = nc.scalar.activation(
                        h2[:, ob, c0 : c0 + cn], psum[:, :cn], act_silu
                    )
                    if ob == 1:
                        base1_marker.append(act)

                _stage(nc, wa, ps, "wa", b1, xb_s, cts_b, ev_base1, wt0=wt_b1)

                # expert tokens: loaded during base compute; explicitly
                # gated on early base1 progress so this 2.2MB transfer never
                # competes with the startup-critical xb/b1 loads
                xg_s = res.tile([P, KA, C], CDT, name="xg_s")
                for k in range(0, KA, 2):
                    dma = nc.sync.dma_start(out=xg_s[:, k : k + 2], in_=xg[:, k : k + 2])
                    add_dep_helper(
                        dma.ins,
                        base1_marker[0].ins,
                        reason="defer xg load past startup window",
                    )

                # Output DMAs rotate over gpsimd/sync; scalar stays
                # dedicated to the weight-tile stream so a queued output
                # never delays the next weight load.
                out_engines = [nc.gpsimd, nc.sync]
                ev_n = [0]

                def _out_dma(dst, o, cn):
                    eng = out_engines[ev_n[0] % 2]
                    ev_n[0] += 1
                    eng.dma_start(out=dst, in_=o[:, :cn])

                def ev_base2(ob, psum, c0, cn):
                    o = yo.tile([P, CHUNK], CDT, name="yo")
                    nc.vector.tensor_copy(out=o[:, :cn], in_=psum[:, :cn])
                    _out_dma(ybase[:, ob, c0 : c0 + cn], o, cn)

                _stage(nc, wb, ps, "wb", b2, h2, cts_b, ev_base2)

                h1 = res.tile([P, KB, C], CDT, name="h1")

                def ev_moe1(ob, psum, c0, cn):
                    nc.scalar.activation(
                        h1[:, ob, c0 : c0 + cn], psum[:, :cn], act_silu
                    )

                _stage(nc, wa, ps, "wa", w1, xg_s, cts_m, ev_moe1)

                wg_s = res.tile([P, C], F32, name="wg_s")
                nc.sync.dma_start(out=wg_s[:], in_=wg[:])

                def ev_moe2(ob, psum, c0, cn):
                    o = yo.tile([P, CHUNK], CDT, name="yo")
                    nc.vector.tensor_tensor(
                        out=o[:, :cn],
                        in0=psum[:, :cn],
                        in1=wg_s[:, c0 : c0 + cn],
                        op=mybir.AluOpType.mult,
                    )
                    _out_dma(ymoe[:, ob, c0 : c0 + cn], o, cn)

                _stage(nc, wb, ps, "wb", w2, h1, cts_m, ev_moe2, cm_last=True)
    nc.compile()
    return nc


_BUILD_CACHE = {}


def _get_program(C):
    if C not in _BUILD_CACHE:
        _BUILD_CACHE[C] = _build(C)
    return _BUILD_CACHE[C]


def _routing(x, router_w):
    """Replicate the reference router bit-for-bit on jax CPU."""
    import jax
    import jax.numpy as jnp

    cpu = jax.devices("cpu")[0]

    def _route(xj, rj):
        logits = xj @ rj
        probs = jax.nn.softmax(logits, axis=-1)
        top_w, top_i = jax.lax.top_k(probs, TOP_K)
        top_w = top_w / jnp.sum(top_w, axis=-1, keepdims=True)
        return top_w, top_i

    with jax.default_device(cpu):
        top_w, top_i = jax.jit(_route)(jnp.asarray(x), jnp.asarray(router_w))
        top_w = np.asarray(top_w)
        top_i = np.asarray(top_i)
    return top_w, top_i


def _as_pkc(w, kb, nb):
    # [K, N] -> [P, nblocks, kblocks, 128]: w[k*128+p, n*128+c] -> [p, n, k, c]
    return np.ascontiguousarray(w.reshape(kb, P, nb, P).transpose(1, 2, 0, 3))


def _as_pit(xt):
    # [R, N] -> [P, R//128, N]: xt[i*128+p, t] -> [p, i, t]
    r, n = xt.shape
    return np.ascontiguousarray(xt.reshape(r // P, P, n).transpose(1, 0, 2))


def _from_pit(y):
    # [P, R//128, N] -> [N, R]
    p, i, n = y.shape
    return y.transpose(2, 1, 0).reshape(n, i * p)


def kernel(hidden_states, router_w, base_w1, base_w2, exp_w1, exp_w2):
    x = np.ascontiguousarray(hidden_states.reshape(T, H), dtype=np.float32)
    top_w, top_i = _routing(x, np.asarray(router_w, dtype=np.float32))

    # per-expert token lists
    idx = []
    wts = []
    for e in range(E):
        rows, slots = np.nonzero(top_i == e)
        idx.append(rows)
        wts.append(top_w[rows, slots].astype(np.float32))
    nmax = max(len(r) for r in idx)
    C = max(((nmax + 63) // 64) * 64, 64)

    nc = _get_program(C)

    xT_c = np.ascontiguousarray(x.T).astype(NP_CDT)  # [H, T]

    b1_dev = _as_pkc(np.asarray(base_w1, np.float32).astype(NP_CDT), KA, FB)
    b2_dev = _as_pkc(np.asarray(base_w2, np.float32).astype(NP_CDT), KB, HB)

    in_maps = []
    for e in range(E):
        n_e = len(idx[e])
        xg_full = np.zeros((H, C), dtype=NP_CDT)
        xg_full[:, :n_e] = xT_c[:, idx[e]]
        wg_full = np.zeros((C,), dtype=np.float32)
        wg_full[:n_e] = wts[e]
        in_maps.append(
            {
                "xg": _as_pit(xg_full),
                "wg": np.ascontiguousarray(np.broadcast_to(wg_full, (P, C))),
                "w1": _as_pkc(
                    np.asarray(exp_w1[e], np.float32).astype(NP_CDT), KA, FB
                ),
                "w2": _as_pkc(
                    np.asarray(exp_w2[e], np.float32).astype(NP_CDT), KB, HB
                ),
                "xb": _as_pit(xT_c[:, e * NB : (e + 1) * NB]),
                "b1": b1_dev,
                "b2": b2_dev,
            }
        )

    res = run_bass_kernel_spmd(nc, in_maps, core_ids=list(range(8)))

    out = np.empty((T, H), dtype=np.float32)
    for e in range(E):
        out[e * NB : (e + 1) * NB] = _from_pit(
            res.results[e]["ybase"].astype(np.float32)
        )
    for e in range(E):
        n_e = len(idx[e])
        ym = _from_pit(res.results[e]["ymoe"].astype(np.float32))[:n_e]
        out[idx[e]] += BETA * ym
    return out.reshape(B, S, H)

